# revision 19
# baseline (speedup 1.0000x reference)
"""Multi-head self-attention (B=2, T=2048, D=1024, H=16) on 8 trn2 NeuronCores.

Sharding: core c handles batch b = c//4 and heads [4*(c%4), 4*(c%4)+4).
Data-parallel over B, tensor-parallel over heads. No on-device collectives:
the o_proj all-reduce (sum of 4 per-core partials per batch) and the final
bias add happen on the host during unshard, as does re-assembly of the
attention-weights tensor (device writes weights transposed per head).

On-device dataflow (everything in "transposed" [feature, token] layout):
  - inputs: x^T [D,T] f32, per-core Wqkv cols [D, 768] f32, Wo rows [256, D]
    f32, and (1-mask)^T pre-tiled to [128, kt, q] bf16.
  - Q^T,K^T = Wqkv_blk^T @ x^T   (fp32r matmuls, [dh,T] layout)
  - V       = x @ Wv             (natural [T,dh] layout, + ones column)
  - S^T[k,q] = K^T.T @ Q^T per (head, 512-wide q block), 16 k-tiles
  - e = exp(S/8) on ScalarE (psum -> sbuf bf16), masked by TT-mult (bf16 2x)
  - AV with ones-augmented V: psum rows 0:64 = O^T_unnorm, row 64 = softmax
    denominator (both accumulate over the 16 k-tiles on the PE)
  - r = 1/den (DVE), broadcast across partitions with a K=1 PE outer product
  - weights_out = e * r (bf16, 2x), O^T = O^T_unnorm * r
  - y = O^T.T @ Wo (fp32r) -> per-core partial output
"""

import os
import sys

import numpy as np

for _p in ("/opt/trn_rl_repo",):
    if _p not in sys.path and os.path.isdir(_p):
        sys.path.insert(0, _p)

import ml_dtypes

import concourse.bacc as bacc
import concourse.bass as bass
import concourse.mybir as mybir
import concourse.tile as tile
from concourse.bass_utils import run_bass_kernel_spmd


B, T, D, H = 2, 2048, 1024, 16
DH = D // H          # 64
NCORES = 8
HPC = 4              # heads per core
DIN = HPC * DH       # 256 o_proj input rows per core
KT = T // 128        # 16 key tiles
QB = T // 512        # 4 query blocks
P = 128

f32 = mybir.dt.float32
f32r = mybir.dt.float32r
bf16 = mybir.dt.bfloat16

Exp = mybir.ActivationFunctionType.Exp

# set by test.py to collect a profile
TRACE = False
LAST_RESULTS = None


def _emit(tc, nc, x_t, wqkv, wo, maskt, wt_o, y_o):
    import contextlib

    ctx = contextlib.ExitStack()
    with ctx:
        # ---------------- persistent pools ----------------
        const_pool = ctx.enter_context(tc.tile_pool(name="const", bufs=1))
        qk_pool = ctx.enter_context(tc.tile_pool(name="qk", bufs=1))
        v_pool = ctx.enter_context(tc.tile_pool(name="v", bufs=1))
        o_pool = ctx.enter_context(tc.tile_pool(name="o", bufs=1))
        wo_pool = ctx.enter_context(tc.tile_pool(name="wo", bufs=1))
        den_pool = ctx.enter_context(tc.tile_pool(name="den", bufs=1))

        ones_row = const_pool.tile([1, P], f32, tag="ones", name="ones")
        nc.vector.memset(ones_row[:], 1.0)

        # Q^T / K^T: 2 tiles each of [128, T]; tile i holds heads 2i, 2i+1
        qt_sb = [qk_pool.tile([P, T], f32r, tag=f"qt{i}", name=f"qt{i}") for i in range(2)]
        kt_sb = [qk_pool.tile([P, T], f32r, tag=f"kt{i}", name=f"kt{i}") for i in range(2)]
        # V (natural layout, per key-tile), 65 cols per head (64 V + ones)
        v_sb = [v_pool.tile([P, HPC * 65], bf16, tag=f"v{i}", name=f"v{i}") for i in range(KT)]
        # o_proj lhsT [d_in, q]; tile i holds heads 2i, 2i+1
        o_sb = [o_pool.tile([P, T], f32r, tag=f"o{i}", name=f"o{i}") for i in range(2)]
        wo_sb = [wo_pool.tile([P, D], f32r, tag=f"wo{i}", name=f"wo{i}") for i in range(2)]
        # per-head reciprocal denominators [1, T]
        r_h = [den_pool.tile([1, T], f32, tag=f"r{i}", name=f"r{i}") for i in range(HPC)]

        for i in range(2):
            nc.sync.dma_start(wo_sb[i][:], wo[i * P:(i + 1) * P, :])

        # ---------------- phase A: qkv projection ----------------
        with (
            tc.tile_pool(name="xt", bufs=1) as xt_pool,
            tc.tile_pool(name="wq", bufs=1) as wq_pool,
            tc.tile_pool(name="psA", bufs=4, space="PSUM") as psA,
        ):
            xt = []
            wq = []
            for i in range(D // P):
                t = xt_pool.tile([P, T], f32r, tag=f"x{i}", name=f"xx{i}")
                nc.sync.dma_start(t[:], x_t[i * P:(i + 1) * P, :])
                xt.append(t)
                w = wq_pool.tile([P, 3 * DIN], f32r, tag=f"w{i}", name=f"w{i}")
                nc.sync.dma_start(w[:], wqkv[i * P:(i + 1) * P, :])
                wq.append(w)
            # Q^T, K^T: out [128 d', 512 t] blocks
            for blk in range(4):  # 0,1 -> Q; 2,3 -> K
                dest = qt_sb[blk] if blk < 2 else kt_sb[blk - 2]
                for tcol in range(4):
                    ps = psA.tile([P, 512], f32, tag="psqk", name="psqk")
                    for i in range(D // P):
                        nc.tensor.matmul(
                            ps[:],
                            wq[i][:, blk * P:(blk + 1) * P],
                            xt[i][:, tcol * 512:(tcol + 1) * 512],
                            start=(i == 0),
                            stop=(i == D // P - 1),
                        )
                    nc.scalar.copy(dest[:, tcol * 512:(tcol + 1) * 512], ps[:])

            # V natural: out [128 t, 256 dh]
            for tt in range(KT):
                ps = psA.tile([P, DIN], f32, tag="psv", name="psv")
                for i in range(D // P):
                    nc.tensor.matmul(
                        ps[:],
                        xt[i][:, tt * P:(tt + 1) * P],
                        wq[i][:, 2 * DIN:3 * DIN],
                        start=(i == 0),
                        stop=(i == D // P - 1),
                    )
                # scatter the 4 heads' 64 cols into 65-col groups
                dst = v_sb[tt][:].rearrange("p (h j) -> p h j", h=HPC)[:, :, 0:DH]
                src = ps[:].rearrange("p (h j) -> p h j", h=HPC)
                nc.scalar.copy(dst, src)
                ones_col = v_sb[tt][:].rearrange("p (h j) -> p h j", h=HPC)[:, :, DH:65]
                nc.vector.memset(ones_col, 1.0)

        # ---------------- phase B: attention ----------------
        with (
            tc.tile_pool(name="mask", bufs=1) as mask_pool,
            tc.tile_pool(name="e", bufs=2) as e_pool,
            tc.tile_pool(name="rbc", bufs=3) as r_pool,
            tc.tile_pool(name="psS", bufs=2, space="PSUM") as psS,
            tc.tile_pool(name="psO", bufs=2, space="PSUM") as psO,
            tc.tile_pool(name="psR", bufs=2, space="PSUM") as psR,
        ):
            maskt_sb = mask_pool.tile([P, KT * T], bf16, tag="mask", name="maskt_sb")
            nc.sync.dma_start(maskt_sb[:], maskt[:, :])
            mask3 = maskt_sb[:].rearrange("p (kt q) -> p kt q", kt=KT)

            for h in range(HPC):
                blk, off = h // 2, (h % 2) * DH
                for qb in range(QB):
                    qsl = slice(qb * 512, (qb + 1) * 512)
                    e_big = e_pool.tile([P, KT * 512], bf16, tag="e", name="e_big")
                    # scores + exp, 2 k-tiles per psum round
                    for r in range(KT // 2):
                        ps = psS.tile([P, 1024], f32, tag="s", name="pss")
                        for j in range(2):
                            kt_i = r * 2 + j
                            nc.tensor.matmul(
                                ps[:, j * 512:(j + 1) * 512],
                                kt_sb[blk][off:off + DH, kt_i * P:(kt_i + 1) * P],
                                qt_sb[blk][off:off + DH, qsl],
                                start=True,
                                stop=True,
                            )
                        nc.scalar.activation(
                            e_big[:, r * 1024:(r + 1) * 1024], ps[:], Exp,
                            scale=1.0 / np.sqrt(DH),
                        )
                    # mask (multiplicative, bf16 2x)
                    e3 = e_big[:].rearrange("p (kt j) -> p kt j", kt=KT)
                    nc.vector.tensor_mul(e3, e3, mask3[:, :, qsl])
                    # AV with ones-augmented V: rows 0:64 = O^T, row 64 = den
                    ps_o = psO.tile([P, 512], f32, tag="av", name="psav")
                    for kt_i in range(KT):
                        nc.tensor.matmul(
                            ps_o[0:65, :],
                            v_sb[kt_i][:, h * 65:(h + 1) * 65],
                            e_big[:, kt_i * 512:(kt_i + 1) * 512],
                            start=(kt_i == 0),
                            stop=(kt_i == KT - 1),
                        )
                    # r = 1/den straight from psum
                    nc.vector.reciprocal(r_h[h][0:1, qsl], ps_o[64:65, :])
                    # broadcast r across partitions via K=1 outer product
                    ps_r = psR.tile([P, 512], f32, tag="rb", name="psrb")
                    nc.tensor.matmul(
                        ps_r[:],
                        ones_row[:],
                        r_h[h][0:1, qsl],
                        start=True,
                        stop=True,
                    )
                    r_sb = r_pool.tile([P, 512], bf16, tag="rsb", name="rsb")
                    nc.vector.tensor_copy(r_sb[:], ps_r[:])
                    # normalized O^T -> o_sb (f32r)
                    nc.vector.tensor_mul(
                        o_sb[blk][off:off + DH, qsl], ps_o[0:DH, :], r_sb[0:DH, :]
                    )
                    # normalized weights (in-place, bf16 2x) -> HBM
                    nc.vector.tensor_mul(
                        e3, e3, r_sb[:, None, :].broadcast_to([P, KT, 512])
                    )
                    nc.sync.dma_start(
                        wt_o[h, qb, :, :], e_big[:]
                    )

        # ---------------- phase C: o_proj ----------------
        with (
            tc.tile_pool(name="y", bufs=4) as y_pool,
            tc.tile_pool(name="psY", bufs=4, space="PSUM") as psY,
        ):
            for qt in range(T // P):
                for half in range(2):
                    ps_y = psY.tile([P, 512], f32, tag="y", name="psy")
                    for i in range(2):
                        nc.tensor.matmul(
                            ps_y[:],
                            o_sb[i][:, qt * P:(qt + 1) * P],
                            wo_sb[i][:, half * 512:(half + 1) * 512],
                            start=(i == 0),
                            stop=(i == 1),
                        )
                    y_t = y_pool.tile([P, 512], f32, tag="yt", name="yt")
                    nc.scalar.copy(y_t[:], ps_y[:])
                    nc.sync.dma_start(
                        y_o[qt * P:(qt + 1) * P, half * 512:(half + 1) * 512], y_t[:]
                    )


def _build_nc():
    nc = bacc.Bacc("TRN2", target_bir_lowering=False, debug=False, num_devices=NCORES)
    x_t = nc.declare_dram_parameter("x_t", [D, T], f32r, isOutput=False)
    wqkv = nc.declare_dram_parameter("wqkv", [D, 3 * DIN], f32r, isOutput=False)
    wo = nc.declare_dram_parameter("wo", [DIN, D], f32r, isOutput=False)
    maskt = nc.declare_dram_parameter("maskt", [P, KT * T], bf16, isOutput=False)
    # weights, transposed per head: wt_o[h, qb, p, kt*512 + j]
    #   = softmax_weights[4*(c%4)+h, q = qb*512 + j, k = kt*128 + p]
    wt_o = nc.declare_dram_parameter("wt_o", [HPC, QB, P, KT * 512], bf16, isOutput=True)
    y_o = nc.declare_dram_parameter("y_o", [T, D], f32, isOutput=True)
    with tile.TileContext(nc) as tc:
        _emit(tc, nc, x_t, wqkv, wo, maskt, wt_o, y_o)
    nc.compile()
    return nc


_NC = None


def _get_nc():
    global _NC
    if _NC is None:
        _NC = _build_nc()
    return _NC


def _host_inputs(x, attn_mask, Wqkv):
    """Per-batch host prep shared by 4 cores each."""
    x_t = [np.ascontiguousarray(x[b].T, dtype=np.float32) for b in range(B)]
    maskt = []
    for b in range(B):
        m01 = (~attn_mask[b]).astype(ml_dtypes.bfloat16)  # [q, k] 1=keep
        # -> [p, kt, q] (k = kt*128 + p)
        mt = np.ascontiguousarray(
            m01.T.reshape(KT, P, T).transpose(1, 0, 2).reshape(P, KT * T)
        )
        maskt.append(mt)
    return x_t, maskt


def kernel(x, attn_mask, Wqkv, Wo, bo):
    x = np.asarray(x, dtype=np.float32)
    attn_mask = np.asarray(attn_mask, dtype=bool)
    Wqkv = np.asarray(Wqkv, dtype=np.float32)
    Wo = np.asarray(Wo, dtype=np.float32)
    bo = np.asarray(bo, dtype=np.float32)

    nc = _get_nc()
    x_t, maskt = _host_inputs(x, attn_mask, Wqkv)

    in_maps = []
    for c in range(NCORES):
        b, g = divmod(c, 4)
        h0 = HPC * g
        cols = np.concatenate(
            [
                Wqkv[:, h0 * DH: h0 * DH + DIN],
                Wqkv[:, D + h0 * DH: D + h0 * DH + DIN],
                Wqkv[:, 2 * D + h0 * DH: 2 * D + h0 * DH + DIN],
            ],
            axis=1,
        )
        in_maps.append(
            {
                "x_t": x_t[b],
                "wqkv": np.ascontiguousarray(cols),
                "wo": np.ascontiguousarray(Wo[h0 * DH: h0 * DH + DIN, :]),
                "maskt": maskt[b],
            }
        )

    global LAST_RESULTS
    res = run_bass_kernel_spmd(nc, in_maps, list(range(NCORES)), trace=TRACE)
    LAST_RESULTS = res

    weights = np.empty((B, H, T, T), dtype=np.float32)
    out = np.zeros((B, T, D), dtype=np.float32)
    for c, r in enumerate(res.results):
        b, g = divmod(c, 4)
        h0 = HPC * g
        wt = np.asarray(r["wt_o"])  # [HPC, QB, P, KT*512] bf16
        for hl in range(HPC):
            # [qb, p, kt, j] -> [q=qb*512+j, k=kt*128+p]
            a = wt[hl].reshape(QB, P, KT, 512).astype(np.float32)
            weights[b, h0 + hl] = a.transpose(0, 3, 2, 1).reshape(T, T)
        out[b] += np.asarray(r["y_o"], dtype=np.float32)
    out += bo[None, None, :]
    return out, weights
